# revision 1
# baseline (speedup 1.0000x reference)
"""QRNN fo-pooling kernel for Trainium2 (Bass/Tile), batch-sharded across 8 cores.

Reference computation (per (b, h) element, sequential over t):
    F, Z, O = split(Y, 3, axis=2); F = sigmoid(F); Z = tanh(Z); O = sigmoid(O)
    c_t = F_t * c_{t-1} + (1 - F_t) * Z_t
    h_t = O_t * c_t
    out = concat([init_h, h], axis=0)

Mapping: the recurrence is a first-order linear scan -> DVE tensor_tensor_scan
(state = data0 * state + data1 along the free dim, fp32 state). Time must be on
the free dim, so raw F/Z are PE-transposed [t,h]->[h,t] (fp32 transpose mode),
activations run on ACT reading PSUM directly (doubling as the PSUM drain), the
scan runs per (b, h-block) over the full T=512, and c is PE-transposed back to
natural [t,h] layout where it is multiplied by sigmoid(O) and stored with
contiguous 512B rows.
"""

import numpy as np

import concourse.bacc as bacc
import concourse.bass as bass
import concourse.mybir as mybir
import concourse.tile as tile
from concourse.bass_utils import run_bass_kernel_spmd
from concourse.masks import make_identity


T, B, H = 512, 32, 1024
LOADB = 3
CB = 3
N_CORES = 8
BS = B // N_CORES  # batches per core
P = 128
HB = H // P  # h-blocks per core
TJ = T // P  # t-chunks

FP32 = mybir.dt.float32

_nc_cache = []


def _build_bass(repeat: int = 1) -> bass.Bass:
    nc = bacc.Bacc("TRN2", target_bir_lowering=False)
    y = nc.declare_dram_parameter("Y", [T, BS, 3 * H], FP32, isOutput=False)
    init_c = nc.declare_dram_parameter("init_c", [1, BS, H], FP32, isOutput=False)
    init_h = nc.declare_dram_parameter("init_h", [1, BS, H], FP32, isOutput=False)
    out = nc.declare_dram_parameter("out", [T + 1, BS, H], FP32, isOutput=True)

    with tile.TileContext(nc) as tc:
        with (
            tc.tile_pool(name="sb", bufs=3) as sb,
            tc.tile_pool(name="psum", bufs=2, space="PSUM") as psum,
            tc.tile_pool(name="singles", bufs=1) as singles,
        ):
            ident = singles.tile([P, P], FP32)
            make_identity(nc, ident)

            # out[0] = init_h[0] (row 0 of the output is the initial h)
            nc.sync.dma_start(out=out[0, :, :], in_=init_h[0, :, :])

            # [t, b, c] -> [p, j, b, c] with t = j*128 + p
            yr = y[:, :, :].rearrange("(j p) b c -> p j b c", p=P)
            outr = out[1 : T + 1, :, :].rearrange("(j p) b h -> p j b h", p=P)
            # all initial states in one load: [p=h%128, hb, b]
            ic_all = singles.tile([P, BS, HB], FP32)
            nc.sync.dma_start(
                out=ic_all,
                in_=init_c[0, :, :].rearrange("b (hb p) -> p b hb", p=P),
            )

            for rep in range(repeat):
              for hb in range(HB):
                h0 = hb * P

                for b in range(BS):
                    # natural-layout loads: [p=t%128, j=t//128, h] (512B rows),
                    # issued on three different queues to spread SEQ cost
                    f_raw = sb.tile([P, TJ, P], FP32, tag="f_raw", bufs=LOADB)
                    z_raw = sb.tile([P, TJ, P], FP32, tag="z_raw", bufs=LOADB)
                    o_raw = sb.tile([P, TJ, P], FP32, tag="o_raw", bufs=LOADB)
                    nc.sync.dma_start(out=f_raw, in_=yr[:, :, b, h0 : h0 + P])
                    nc.sync.dma_start(out=z_raw, in_=yr[:, :, b, H + h0 : H + h0 + P])
                    nc.gpsimd.dma_start(
                        out=o_raw, in_=yr[:, :, b, 2 * H + h0 : 2 * H + h0 + P]
                    )

                    # PE transpose raw F and Z: [t, h] -> [h, t], PSUM cols = t
                    ps_f = psum.tile([P, T], FP32, tag="ps_f")
                    ps_z = psum.tile([P, T], FP32, tag="ps_z")
                    for j in range(TJ):
                        nc.tensor.transpose(
                            ps_f[:, j * P : (j + 1) * P], f_raw[:, j, :], ident
                        )
                        nc.tensor.transpose(
                            ps_z[:, j * P : (j + 1) * P], z_raw[:, j, :], ident
                        )

    # ACT reads PSUM, writes SBUF (doubles as PSUM drain):
                    # s_neg = sigmoid(-F_raw) = 1 - f ; zt = tanh(Z_raw)
                    s_neg = sb.tile([P, T], FP32, tag="s_neg", bufs=CB)
                    nc.scalar.activation(
                        s_neg, ps_f[:, :], mybir.ActivationFunctionType.Sigmoid,
                        scale=-1.0,
                    )
                    zt = sb.tile([P, T], FP32, tag="zt", bufs=CB)
                    nc.scalar.activation(
                        zt, ps_z[:, :], mybir.ActivationFunctionType.Tanh
                    )

                    # f = 1 - s_neg on the (otherwise idle) gpsimd engine
                    f_t = sb.tile([P, T], FP32, tag="f_t", bufs=CB)
                    nc.gpsimd.tensor_scalar(
                        f_t, s_neg, -1.0, 1.0,
                        op0=mybir.AluOpType.mult, op1=mybir.AluOpType.add,
                    )
                    # zf = (1 - f) * tanh(z) = s_neg * zt
                    zf = sb.tile([P, T], FP32, tag="zf", bufs=CB)
                    nc.vector.tensor_mul(zf, zt, s_neg)

                    # the recurrence: c[:, t] = f[:, t] * c[:, t-1] + zf[:, t]
                    c_t = sb.tile([P, T], FP32, tag="c_t", bufs=CB)
                    nc.vector.tensor_tensor_scan(
                        c_t, f_t, zf, initial=ic_all[:, b, hb : hb + 1],
                        op0=mybir.AluOpType.mult, op1=mybir.AluOpType.add,
                    )

                    # transpose c back to natural layout: [h, t] -> [p=t%128, j, h]
                    ps_c = psum.tile([P, T], FP32, tag="ps_c")
                    for j in range(TJ):
                        nc.tensor.transpose(
                            ps_c[:, j * P : (j + 1) * P],
                            c_t[:, j * P : (j + 1) * P],
                            ident,
                        )

    # h = sigmoid(O_raw) * c, all in natural layout
                    o_sig = sb.tile([P, TJ, P], FP32, tag="o_sig")
                    nc.scalar.activation(
                        o_sig, o_raw[:, :, :], mybir.ActivationFunctionType.Sigmoid
                    )
                    h_out = sb.tile([P, TJ * P], FP32, tag="h_out")
                    nc.vector.tensor_mul(
                        h_out, o_sig.rearrange("p j h -> p (j h)"), ps_c[:, :]
                    )

                    # stores go out on the Activation HWDGE queue to keep the
                    # SP sequencer free for load issue
                    nc.scalar.dma_start(
                        out=outr[:, :, b, h0 : h0 + P], in_=h_out
                    )
    nc.compile()
    return nc


def _get_nc() -> bass.Bass:
    if not _nc_cache:
        _nc_cache.append(_build_bass())
    return _nc_cache[0]


def kernel(Y: np.ndarray, init_c: np.ndarray, init_h: np.ndarray) -> np.ndarray:
    Y = np.ascontiguousarray(np.asarray(Y, dtype=np.float32))
    init_c = np.ascontiguousarray(np.asarray(init_c, dtype=np.float32))
    init_h = np.ascontiguousarray(np.asarray(init_h, dtype=np.float32))

    in_maps = []
    for k in range(N_CORES):
        sl = slice(k * BS, (k + 1) * BS)
        in_maps.append(
            {
                "Y": np.ascontiguousarray(Y[:, sl, :]),
                "init_c": np.ascontiguousarray(init_c[:, sl, :]),
                "init_h": np.ascontiguousarray(init_h[:, sl, :]),
            }
        )

    nc = _get_nc()
    res = run_bass_kernel_spmd(nc, in_maps, core_ids=list(range(N_CORES)))
    return np.concatenate([r["out"] for r in res.results], axis=1)



# revision 3
# speedup vs baseline: 3.3778x; 3.3778x over previous
"""QRNN fo-pooling kernel for Trainium2 (Bass/Tile), batch-sharded across 8 cores.

Reference computation (per (b, h) element, sequential over t):
    F, Z, O = split(Y, 3, axis=2); F = sigmoid(F); Z = tanh(Z); O = sigmoid(O)
    c_t = F_t * c_{t-1} + (1 - F_t) * Z_t
    h_t = O_t * c_t
    out = concat([init_h, h], axis=0)

Mapping: the recurrence is a first-order linear scan -> DVE tensor_tensor_scan
(state = data0 * state + data1 along the free dim, fp32 state). Time must be on
the free dim, so raw F/Z are PE-transposed [t,h]->[h,t] (fp32 transpose mode),
activations run on ACT reading PSUM directly (doubling as the PSUM drain), the
scan runs per (b, h-block) over the full T=512, and c is PE-transposed back to
natural [t,h] layout where it is multiplied by sigmoid(O) and stored with
contiguous 512B rows.
"""

import numpy as np

import concourse.bacc as bacc
import concourse.bass as bass
import concourse.mybir as mybir
import concourse.tile as tile
from concourse.bass_utils import run_bass_kernel_spmd
from concourse.masks import make_identity


T, B, H = 512, 32, 1024
LOADB = 3
CB = 3
N_CORES = 8
BS = B // N_CORES  # batches per core
P = 128
HB = H // P  # h-blocks per core
TJ = T // P  # t-chunks

FP32 = mybir.dt.float32

_nc_cache = []


def _build_bass(repeat: int = 1) -> bass.Bass:
    nc = bacc.Bacc("TRN2", target_bir_lowering=False)
    y = nc.declare_dram_parameter("Y", [T, BS, 3 * H], FP32, isOutput=False)
    init_c = nc.declare_dram_parameter("init_c", [1, BS, H], FP32, isOutput=False)
    init_h = nc.declare_dram_parameter("init_h", [1, BS, H], FP32, isOutput=False)
    out = nc.declare_dram_parameter("out", [T + 1, BS, H], FP32, isOutput=True)

    with tile.TileContext(nc) as tc:
        with (
            tc.tile_pool(name="sb", bufs=3) as sb,
            tc.tile_pool(name="psum", bufs=2, space="PSUM") as psum,
            tc.tile_pool(name="singles", bufs=1) as singles,
        ):
            ident = singles.tile([P, P], FP32)
            make_identity(nc, ident)

            # out[0] = init_h[0] (row 0 of the output is the initial h)
            nc.sync.dma_start(out=out[0, :, :], in_=init_h[0, :, :])

            # [t, b, c] -> [p, j, b, c] with t = j*128 + p
            yr = y[:, :, :].rearrange("(j p) b c -> p j b c", p=P)
            outr = out[1 : T + 1, :, :].rearrange("(j p) b h -> p j b h", p=P)
            # all initial states in one load: [p=h%128, hb, b]
            ic_all = singles.tile([P, BS, HB], FP32)
            nc.sync.dma_start(
                out=ic_all,
                in_=init_c[0, :, :].rearrange("b (hb p) -> p b hb", p=P),
            )

            def _rep_body():
              for hb in range(HB):
                h0 = hb * P

                for b in range(BS):
                    # natural-layout loads: [p=t%128, j=t//128, h] (512B rows),
                    # issued on three different queues to spread SEQ cost
                    f_raw = sb.tile([P, TJ, P], FP32, tag="f_raw", bufs=LOADB)
                    z_raw = sb.tile([P, TJ, P], FP32, tag="z_raw", bufs=LOADB)
                    o_raw = sb.tile([P, TJ, P], FP32, tag="o_raw", bufs=LOADB)
                    nc.sync.dma_start(out=f_raw, in_=yr[:, :, b, h0 : h0 + P])
                    nc.sync.dma_start(out=z_raw, in_=yr[:, :, b, H + h0 : H + h0 + P])
                    nc.gpsimd.dma_start(
                        out=o_raw, in_=yr[:, :, b, 2 * H + h0 : 2 * H + h0 + P]
                    )

                    # PE transpose raw F and Z: [t, h] -> [h, t], PSUM cols = t
                    ps_f = psum.tile([P, T], FP32, tag="ps_f")
                    ps_z = psum.tile([P, T], FP32, tag="ps_z")
                    for j in range(TJ):
                        nc.tensor.transpose(
                            ps_f[:, j * P : (j + 1) * P], f_raw[:, j, :], ident
                        )
                        nc.tensor.transpose(
                            ps_z[:, j * P : (j + 1) * P], z_raw[:, j, :], ident
                        )

    # ACT reads PSUM, writes SBUF (doubles as PSUM drain):
                    # s_neg = sigmoid(-F_raw) = 1 - f ; zt = tanh(Z_raw)
                    s_neg = sb.tile([P, T], FP32, tag="s_neg", bufs=CB)
                    nc.scalar.activation(
                        s_neg, ps_f[:, :], mybir.ActivationFunctionType.Sigmoid,
                        scale=-1.0,
                    )
                    zt = sb.tile([P, T], FP32, tag="zt", bufs=CB)
                    nc.scalar.activation(
                        zt, ps_z[:, :], mybir.ActivationFunctionType.Tanh
                    )

                    # f = 1 - s_neg on the (otherwise idle) gpsimd engine
                    f_t = sb.tile([P, T], FP32, tag="f_t", bufs=CB)
                    nc.gpsimd.tensor_scalar(
                        f_t, s_neg, -1.0, 1.0,
                        op0=mybir.AluOpType.mult, op1=mybir.AluOpType.add,
                    )
                    # zf = (1 - f) * tanh(z) = s_neg * zt
                    zf = sb.tile([P, T], FP32, tag="zf", bufs=CB)
                    nc.vector.tensor_mul(zf, zt, s_neg)

                    # the recurrence: c[:, t] = f[:, t] * c[:, t-1] + zf[:, t]
                    c_t = sb.tile([P, T], FP32, tag="c_t", bufs=CB)
                    nc.vector.tensor_tensor_scan(
                        c_t, f_t, zf, initial=ic_all[:, b, hb : hb + 1],
                        op0=mybir.AluOpType.mult, op1=mybir.AluOpType.add,
                    )

                    # transpose c back to natural layout: [h, t] -> [p=t%128, j, h]
                    ps_c = psum.tile([P, T], FP32, tag="ps_c")
                    for j in range(TJ):
                        nc.tensor.transpose(
                            ps_c[:, j * P : (j + 1) * P],
                            c_t[:, j * P : (j + 1) * P],
                            ident,
                        )

    # h = sigmoid(O_raw) * c, all in natural layout
                    o_sig = sb.tile([P, TJ, P], FP32, tag="o_sig")
                    nc.scalar.activation(
                        o_sig, o_raw[:, :, :], mybir.ActivationFunctionType.Sigmoid
                    )
                    h_out = sb.tile([P, TJ * P], FP32, tag="h_out")
                    nc.vector.tensor_mul(
                        h_out, o_sig.rearrange("p j h -> p (j h)"), ps_c[:, :]
                    )

                    # stores go out on the Activation HWDGE queue to keep the
                    # SP sequencer free for load issue
                    nc.scalar.dma_start(
                        out=outr[:, :, b, h0 : h0 + P], in_=h_out
                    )

            if repeat == 1:
                _rep_body()
            else:
                # timing mode: hardware loop keeps the NEFF size constant in
                # `repeat`, so two loop bounds can be wall-clock diffed
                with tc.For_i(0, repeat, 1):
                    _rep_body()
    nc.compile()
    return nc


def _get_nc() -> bass.Bass:
    if not _nc_cache:
        _nc_cache.append(_build_bass())
    return _nc_cache[0]


def kernel(Y: np.ndarray, init_c: np.ndarray, init_h: np.ndarray) -> np.ndarray:
    Y = np.ascontiguousarray(np.asarray(Y, dtype=np.float32))
    init_c = np.ascontiguousarray(np.asarray(init_c, dtype=np.float32))
    init_h = np.ascontiguousarray(np.asarray(init_h, dtype=np.float32))

    in_maps = []
    for k in range(N_CORES):
        sl = slice(k * BS, (k + 1) * BS)
        in_maps.append(
            {
                "Y": np.ascontiguousarray(Y[:, sl, :]),
                "init_c": np.ascontiguousarray(init_c[:, sl, :]),
                "init_h": np.ascontiguousarray(init_h[:, sl, :]),
            }
        )

    nc = _get_nc()
    res = run_bass_kernel_spmd(nc, in_maps, core_ids=list(range(N_CORES)))
    return np.concatenate([r["out"] for r in res.results], axis=1)



# revision 5
# speedup vs baseline: 3.4685x; 1.0268x over previous
"""QRNN fo-pooling kernel for Trainium2 (Bass/Tile), batch-sharded across 8 cores.

Reference computation (per (b, h) element, sequential over t):
    F, Z, O = split(Y, 3, axis=2); F = sigmoid(F); Z = tanh(Z); O = sigmoid(O)
    c_t = F_t * c_{t-1} + (1 - F_t) * Z_t
    h_t = O_t * c_t
    out = concat([init_h, h], axis=0)

v2 design (slab loads + batched engines), per batch b and t-chunk j:
  - one HWDGE load pulls the F+Z halves of Y[j*128:(j+1)*128, b, :] as a
    [128, 2048] slab: 8KB contiguous per partition (vs 512B runs), 3x fewer
    HWDGE dispatches than per-gate loads
  - PE transposes 128x128 chunks into ps_f/ps_z PSUM tiles [128, 1024]
    (hb-major), ACT drains them with N=1024 activations (sigmoid(-F), tanh)
  - Pool computes f = 1 - s_neg, DVE computes zf = s_neg * zt, both writing
    full-b tensors [128, HB, TJ, 128] so the recurrence runs as ONE
    tensor_tensor_scan of N=512 per (b, hb) (free dims (j, t) iterate in
    exact time order)
  - phase 3 per (b, j): PE transposes c back to natural layout in [128,512]
    chunks, O is loaded late (4KB runs), ACT sigmoids it, DVE multiplies,
    and the store goes out through SWDGE (gpsimd) with 2KB contiguous rows,
    keeping both HWDGE rings free for loads
"""

import numpy as np

import concourse.bacc as bacc
import concourse.bass as bass
import concourse.mybir as mybir
import concourse.tile as tile
from concourse.bass_utils import run_bass_kernel_spmd
from concourse.masks import make_identity


T, B, H = 512, 32, 1024
N_CORES = 8
BS = B // N_CORES  # batches per core
P = 128
HB = H // P  # h-blocks
TJ = T // P  # t-chunks

FP32 = mybir.dt.float32

_nc_cache = []


def _build_bass(repeat: int = 1) -> bass.Bass:
    nc = bacc.Bacc("TRN2", target_bir_lowering=False)
    y = nc.declare_dram_parameter("Y", [T, BS, 3 * H], FP32, isOutput=False)
    init_c = nc.declare_dram_parameter("init_c", [1, BS, H], FP32, isOutput=False)
    init_h = nc.declare_dram_parameter("init_h", [1, BS, H], FP32, isOutput=False)
    out = nc.declare_dram_parameter("out", [T + 1, BS, H], FP32, isOutput=True)

    with tile.TileContext(nc) as tc:
        with (
            tc.tile_pool(name="sb", bufs=3) as sb,
            tc.tile_pool(name="psum", bufs=2, space="PSUM") as psum,
            tc.tile_pool(name="singles", bufs=1) as singles,
        ):
            ident = singles.tile([P, P], FP32)
            make_identity(nc, ident)

            # out[0] = init_h[0] (row 0 of the output is the initial h)
            nc.sync.dma_start(out=out[0, :, :], in_=init_h[0, :, :])

            # [t, b, c] -> [p, j, b, c] with t = j*128 + p
            yr = y[:, :, :].rearrange("(j p) b c -> p j b c", p=P)
            outr = out[1 : T + 1, :, :].rearrange("(j p) b h -> p j b h", p=P)
            # all initial states in one load: [p=h%128, b, hb]
            ic_all = singles.tile([P, BS, HB], FP32)
            nc.sync.dma_start(
                out=ic_all,
                in_=init_c[0, :, :].rearrange("b (hb p) -> p b hb", p=P),
            )

            def _rep_body():
                for b in range(BS):
                    # full-b gate tensors: [p=h%128, hb, j, t%128]
                    f_t = sb.tile([P, HB, TJ, P], FP32, tag="f_t", bufs=2)
                    zf = sb.tile([P, HB, TJ, P], FP32, tag="zf", bufs=2)
                    c_t = sb.tile([P, HB, TJ, P], FP32, tag="c_t", bufs=2)

                    # phase 1 (per j): load FZ slab, transpose, activations
                    for j in range(TJ):
                        fz = sb.tile([P, 2 * H], FP32, tag="fz", bufs=3)
                        nc.sync.dma_start(out=fz, in_=yr[:, j, b, 0 : 2 * H])

                        ps_f = psum.tile([P, H], FP32, tag="ps_f", bufs=2)
                        ps_z = psum.tile([P, H], FP32, tag="ps_z", bufs=1)
                        for hb in range(HB):
                            nc.tensor.transpose(
                                ps_f[:, hb * P : (hb + 1) * P],
                                fz[:, hb * P : (hb + 1) * P],
                                ident,
                            )
                        for hb in range(HB):
                            nc.tensor.transpose(
                                ps_z[:, hb * P : (hb + 1) * P],
                                fz[:, H + hb * P : H + (hb + 1) * P],
                                ident,
                            )

                        # ACT drains PSUM: s_neg = 1 - sigmoid(F), zt = tanh(Z)
                        s_neg = sb.tile([P, H], FP32, tag="s_neg", bufs=3)
                        nc.scalar.activation(
                            s_neg, ps_f, mybir.ActivationFunctionType.Sigmoid,
                            scale=-1.0,
                        )
                        zt = sb.tile([P, H], FP32, tag="zt", bufs=3)
                        nc.scalar.activation(
                            zt, ps_z, mybir.ActivationFunctionType.Tanh
                        )

                        sr = s_neg.rearrange("p (hb t) -> p hb t", hb=HB)
                        zr = zt.rearrange("p (hb t) -> p hb t", hb=HB)
                        # f = 1 - s_neg on Pool; zf = s_neg * tanh(z) on DVE
                        nc.gpsimd.tensor_scalar(
                            f_t[:, :, j, :], sr, -1.0, 1.0,
                            op0=mybir.AluOpType.mult, op1=mybir.AluOpType.add,
                        )
                        nc.vector.tensor_mul(zf[:, :, j, :], zr, sr)

                    # phase 2: the recurrence, one scan per (b, hb) over all T
                    for hb in range(HB):
                        nc.vector.tensor_tensor_scan(
                            c_t[:, hb, :, :].rearrange("p j t -> p (j t)"),
                            f_t[:, hb, :, :].rearrange("p j t -> p (j t)"),
                            zf[:, hb, :, :].rearrange("p j t -> p (j t)"),
                            initial=ic_all[:, b, hb : hb + 1],
                            op0=mybir.AluOpType.mult,
                            op1=mybir.AluOpType.add,
                        )

                    # phase 3 (per j): c back to natural layout, h = sigmoid(O)*c
                    for j in range(TJ):
                        osl = sb.tile([P, H], FP32, tag="osl", bufs=3)
                        nc.sync.dma_start(out=osl, in_=yr[:, j, b, 2 * H : 3 * H])
                        o_sig = sb.tile([P, H], FP32, tag="o_sig", bufs=3)
                        nc.scalar.activation(
                            o_sig, osl, mybir.ActivationFunctionType.Sigmoid
                        )

                        for half in range(2):
                            ps_c = psum.tile([P, H // 2], FP32, tag="ps_c", bufs=2)
                            for hh in range(HB // 2):
                                hb = half * (HB // 2) + hh
                                nc.tensor.transpose(
                                    ps_c[:, hh * P : (hh + 1) * P],
                                    c_t[:, hb, j, :],
                                    ident,
                                )
                            h_out = sb.tile([P, H // 2], FP32, tag="h_out", bufs=4)
                            nc.vector.tensor_mul(
                                h_out,
                                o_sig[:, half * (H // 2) : (half + 1) * (H // 2)],
                                ps_c,
                            )
                            # store via SWDGE: 2KB contiguous rows, keeps the
                            # HWDGE rings free for loads
                            nc.gpsimd.dma_start(
                                out=outr[
                                    :, j, b,
                                    half * (H // 2) : (half + 1) * (H // 2),
                                ],
                                in_=h_out,
                            )

            if repeat == 1:
                _rep_body()
            else:
                # timing mode: hardware loop keeps the NEFF size constant in
                # `repeat`, so two loop bounds can be wall-clock diffed
                with tc.For_i(0, repeat, 1):
                    _rep_body()
    nc.compile()
    return nc


def _get_nc() -> bass.Bass:
    if not _nc_cache:
        _nc_cache.append(_build_bass())
    return _nc_cache[0]


def kernel(Y: np.ndarray, init_c: np.ndarray, init_h: np.ndarray) -> np.ndarray:
    Y = np.ascontiguousarray(np.asarray(Y, dtype=np.float32))
    init_c = np.ascontiguousarray(np.asarray(init_c, dtype=np.float32))
    init_h = np.ascontiguousarray(np.asarray(init_h, dtype=np.float32))

    in_maps = []
    for k in range(N_CORES):
        sl = slice(k * BS, (k + 1) * BS)
        in_maps.append(
            {
                "Y": np.ascontiguousarray(Y[:, sl, :]),
                "init_c": np.ascontiguousarray(init_c[:, sl, :]),
                "init_h": np.ascontiguousarray(init_h[:, sl, :]),
            }
        )

    nc = _get_nc()
    res = run_bass_kernel_spmd(nc, in_maps, core_ids=list(range(N_CORES)))
    return np.concatenate([r["out"] for r in res.results], axis=1)


# revision 9
# speedup vs baseline: 3.6164x; 1.0426x over previous
"""QRNN fo-pooling kernel for Trainium2 (Bass/Tile), batch-sharded across 8 cores.

Reference computation (per (b, h) element, sequential over t):
    F, Z, O = split(Y, 3, axis=2); F = sigmoid(F); Z = tanh(Z); O = sigmoid(O)
    c_t = F_t * c_{t-1} + (1 - F_t) * Z_t
    h_t = O_t * c_t
    out = concat([init_h, h], axis=0)

v2 design (slab loads + batched engines), per batch b and t-chunk j:
  - one HWDGE load pulls the F+Z halves of Y[j*128:(j+1)*128, b, :] as a
    [128, 2048] slab: 8KB contiguous per partition (vs 512B runs), 3x fewer
    HWDGE dispatches than per-gate loads
  - PE transposes 128x128 chunks into ps_f/ps_z PSUM tiles [128, 1024]
    (hb-major), ACT drains them with N=1024 activations (sigmoid(-F), tanh)
  - Pool computes f = 1 - s_neg, DVE computes zf = s_neg * zt, both writing
    full-b tensors [128, HB, TJ, 128] so the recurrence runs as ONE
    tensor_tensor_scan of N=512 per (b, hb) (free dims (j, t) iterate in
    exact time order)
  - phase 3 per (b, j): PE transposes c back to natural layout in [128,512]
    chunks, O is loaded late (4KB runs), ACT sigmoids it, DVE multiplies,
    and the store goes out through SWDGE (gpsimd) with 2KB contiguous rows,
    keeping both HWDGE rings free for loads
"""

import numpy as np

import concourse.bacc as bacc
import concourse.bass as bass
import concourse.mybir as mybir
import concourse.tile as tile
from concourse.bass_utils import run_bass_kernel_spmd
from concourse.masks import make_identity


T, B, H = 512, 32, 1024
N_CORES = 8
BS = B // N_CORES  # batches per core
P = 128
HB = H // P  # h-blocks
TJ = T // P  # t-chunks

FP32 = mybir.dt.float32

_nc_cache = []


def _build_bass(repeat: int = 1, dma_only: bool = False) -> bass.Bass:
    nc = bacc.Bacc("TRN2", target_bir_lowering=False)
    y = nc.declare_dram_parameter("Y", [T, BS, 3 * H], FP32, isOutput=False)
    init_c = nc.declare_dram_parameter("init_c", [1, BS, H], FP32, isOutput=False)
    init_h = nc.declare_dram_parameter("init_h", [1, BS, H], FP32, isOutput=False)
    out = nc.declare_dram_parameter("out", [T + 1, BS, H], FP32, isOutput=True)

    with tile.TileContext(nc) as tc:
        with (
            tc.tile_pool(name="sb", bufs=3) as sb,
            tc.tile_pool(name="psum", bufs=2, space="PSUM") as psum,
            tc.tile_pool(name="singles", bufs=1) as singles,
        ):
            ident = singles.tile([P, P], FP32)
            make_identity(nc, ident)

            # out[0] = init_h[0] (row 0 of the output is the initial h)
            nc.sync.dma_start(out=out[0, :, :], in_=init_h[0, :, :])

            # [t, b, c] -> [p, j, b, c] with t = j*128 + p
            yr = y[:, :, :].rearrange("(j p) b c -> p j b c", p=P)
            outr = out[1 : T + 1, :, :].rearrange("(j p) b h -> p j b h", p=P)
            # all initial states in one load: [p=h%128, b, hb]
            ic_all = singles.tile([P, BS, HB], FP32)
            nc.sync.dma_start(
                out=ic_all,
                in_=init_c[0, :, :].rearrange("b (hb p) -> p b hb", p=P),
            )

            def _dma_only_body():
                # measurement probe: identical DMA traffic, no compute
                zero = singles.tile([P, H], FP32, tag="zero")
                nc.vector.memset(zero, 0.0)
                for b in range(BS):
                    for j in range(TJ):
                        fz = sb.tile([P, 2 * H], FP32, tag="fz", bufs=3)
                        nc.sync.dma_start(out=fz, in_=yr[:, j, b, 0 : 2 * H])
                        osl = sb.tile([P, H], FP32, tag="osl", bufs=3)
                        nc.sync.dma_start(out=osl, in_=yr[:, j, b, 2 * H : 3 * H])
                        for half in range(2):
                            nc.gpsimd.dma_start(
                                out=outr[
                                    :, j, b,
                                    half * (H // 2) : (half + 1) * (H // 2),
                                ],
                                in_=zero[:, half * (H // 2) : (half + 1) * (H // 2)],
                            )

            def _rep_body():
                if dma_only:
                    _dma_only_body()
                    return
                for b in range(BS):
                    # full-b gate tensors: [p=h%128, hb, j, t%128]
                    f_t = sb.tile([P, HB, TJ, P], FP32, tag="f_t", bufs=2)
                    zf = sb.tile([P, HB, TJ, P], FP32, tag="zf", bufs=2)
                    c_t = sb.tile([P, HB, TJ, P], FP32, tag="c_t", bufs=2)

                    for j in range(TJ):
                        # phase 1: load F/Z slabs (4KB contiguous rows),
                        # prefetch O, transpose, activations
                        fsl = sb.tile([P, H], FP32, tag="fsl", bufs=3)
                        nc.sync.dma_start(out=fsl, in_=yr[:, j, b, 0:H])
                        zsl = sb.tile([P, H], FP32, tag="zsl", bufs=3)
                        nc.sync.dma_start(out=zsl, in_=yr[:, j, b, H : 2 * H])
                        osl = sb.tile([P, H], FP32, tag="osl", bufs=3)
                        nc.sync.dma_start(out=osl, in_=yr[:, j, b, 2 * H : 3 * H])

                        ps_f = psum.tile([P, H], FP32, tag="ps_f", bufs=2)
                        ps_z = psum.tile([P, H], FP32, tag="ps_z", bufs=1)
                        for hb in range(HB):
                            nc.tensor.transpose(
                                ps_f[:, hb * P : (hb + 1) * P],
                                fsl[:, hb * P : (hb + 1) * P],
                                ident,
                            )
                        for hb in range(HB):
                            nc.tensor.transpose(
                                ps_z[:, hb * P : (hb + 1) * P],
                                zsl[:, hb * P : (hb + 1) * P],
                                ident,
                            )

                        # ACT drains PSUM: s_neg = 1 - sigmoid(F), zt = tanh(Z)
                        s_neg = sb.tile([P, H], FP32, tag="s_neg", bufs=3)
                        nc.scalar.activation(
                            s_neg, ps_f, mybir.ActivationFunctionType.Sigmoid,
                            scale=-1.0,
                        )
                        zt = sb.tile([P, H], FP32, tag="zt", bufs=3)
                        nc.scalar.activation(
                            zt, ps_z, mybir.ActivationFunctionType.Tanh
                        )
                        # sigmoid(O) early so phase 3 never waits on ACT
                        o_sig = sb.tile([P, H], FP32, tag="o_sig", bufs=3)
                        nc.scalar.activation(
                            o_sig, osl, mybir.ActivationFunctionType.Sigmoid
                        )

                        sr = s_neg.rearrange("p (hb t) -> p hb t", hb=HB)
                        zr = zt.rearrange("p (hb t) -> p hb t", hb=HB)
                        # f = 1 - s_neg on Pool; zf = s_neg * tanh(z) on DVE
                        nc.gpsimd.tensor_scalar(
                            f_t[:, :, j, :], sr, -1.0, 1.0,
                            op0=mybir.AluOpType.mult, op1=mybir.AluOpType.add,
                        )
                        nc.vector.tensor_mul(zf[:, :, j, :], zr, sr)

                        # phase 2: chained chunk scans — c for this t-chunk is
                        # ready as soon as this chunk's gates are, instead of
                        # waiting for the whole sequence
                        for hb in range(HB):
                            nc.vector.tensor_tensor_scan(
                                c_t[:, hb, j, :],
                                f_t[:, hb, j, :],
                                zf[:, hb, j, :],
                                initial=(
                                    ic_all[:, b, hb : hb + 1]
                                    if j == 0
                                    else c_t[:, hb, j - 1, P - 1 : P]
                                ),
                                op0=mybir.AluOpType.mult,
                                op1=mybir.AluOpType.add,
                            )

                        # phase 3: c back to natural layout, h = sigmoid(O)*c
                        for half in range(2):
                            ps_c = psum.tile([P, H // 2], FP32, tag="ps_c", bufs=2)
                            for hh in range(HB // 2):
                                hb = half * (HB // 2) + hh
                                nc.tensor.transpose(
                                    ps_c[:, hh * P : (hh + 1) * P],
                                    c_t[:, hb, j, :],
                                    ident,
                                )
                            h_out = sb.tile([P, H // 2], FP32, tag="h_out", bufs=4)
                            nc.vector.tensor_mul(
                                h_out,
                                o_sig[:, half * (H // 2) : (half + 1) * (H // 2)],
                                ps_c,
                            )
                            # store via SWDGE: 2KB contiguous rows, keeps the
                            # HWDGE rings free for loads
                            nc.gpsimd.dma_start(
                                out=outr[
                                    :, j, b,
                                    half * (H // 2) : (half + 1) * (H // 2),
                                ],
                                in_=h_out,
                            )

            if repeat == 1:
                _rep_body()
            else:
                # timing mode: hardware loop keeps the NEFF size constant in
                # `repeat`, so two loop bounds can be wall-clock diffed
                with tc.For_i(0, repeat, 1):
                    _rep_body()
    nc.compile()
    return nc


def _get_nc() -> bass.Bass:
    if not _nc_cache:
        _nc_cache.append(_build_bass())
    return _nc_cache[0]


def kernel(Y: np.ndarray, init_c: np.ndarray, init_h: np.ndarray) -> np.ndarray:
    Y = np.ascontiguousarray(np.asarray(Y, dtype=np.float32))
    init_c = np.ascontiguousarray(np.asarray(init_c, dtype=np.float32))
    init_h = np.ascontiguousarray(np.asarray(init_h, dtype=np.float32))

    in_maps = []
    for k in range(N_CORES):
        sl = slice(k * BS, (k + 1) * BS)
        in_maps.append(
            {
                "Y": np.ascontiguousarray(Y[:, sl, :]),
                "init_c": np.ascontiguousarray(init_c[:, sl, :]),
                "init_h": np.ascontiguousarray(init_h[:, sl, :]),
            }
        )

    nc = _get_nc()
    res = run_bass_kernel_spmd(nc, in_maps, core_ids=list(range(N_CORES)))
    return np.concatenate([r["out"] for r in res.results], axis=1)
